# revision 50
# baseline (speedup 1.0000x reference)
"""Trainium2 Bass kernel for the Tucker-factorized (TLE) multi-head attention.

Strategy
--------
Data-parallel over batch: 16 batches / 8 cores = 2 batches per core; every
core runs the full per-batch pipeline (no collectives needed).

Host-side prep: the three per-mode factor matrices of each projection are
folded into one dense 768x768 Kronecker matrix.  Rows (for q/k/v) are
permuted to *head-major* order (h1,h2,h3,x,y,z) so each of the 12 heads
occupies a contiguous 64-partition block -- two heads per 128-partition
chunk.  The softmax scale 1/8 is folded into the q matrix/bias.  The o
matrix gets the inverse permutation on its columns.  Weights, x, and the
output travel as bfloat16; psum accumulation stays fp32, keeping rms error
~5e-3, well inside the 2e-2 gate.  bf16 operands run the PE at 1 cycle/row
with fast weight loads.

Device pipeline, per batch:
  1. DMA x [600,768] token-major (bf16), PE-transpose to xT [768,600].
  2. V projected *token-major* directly (xT chunks stationary, Wv moving):
     no V transposes needed; ones column appended for softmax sums.
  3. Per head pair hp: q/kT = W @ xT + b (feature-major); then per query
     half: scores for both heads of the pair issue into the two PE
     row-groups (rows 0-63 / 64-127) back-to-back and one ScalarE exp
     covers both heads' psum banks; PV via lhsT=[V|1]; normalize with a
     fast DVE reciprocal + gpsimd partition-broadcast.
  4. Output projected token-major (yT chunks stationary, Wo moving): bias
     row-add on DVE, DMA out per token chunk as bf16 (host upcasts).
"""

import numpy as np
import ml_dtypes

import concourse.bass as bass
import concourse.tile as tile
from concourse import bacc, mybir
from concourse.bass_utils import run_bass_kernel_spmd
from concourse.masks import make_identity

# ---------------------------------------------------------------- constants
N_CORES = 8
B = 16
BPC = B // N_CORES          # batches per core
P1, P2 = 25, 24
S = P1 * P2                 # 600 tokens
F = 768                     # flattened feature dim
FC = 6                      # feature chunks of 128
ST = 120                    # token tile
NS = S // ST                # 5 token tiles
NH = 300                    # half of the token axis (psum-bank sized)
FH = 384                    # half of the feature axis (psum-bank sized)
H1, H2, H3 = 2, 2, 3
XD = YD = ZD = 4
NHEADS = H1 * H2 * H3       # 12
HD = 64
F32 = mybir.dt.float32
BF16 = mybir.dt.bfloat16
NPBF16 = ml_dtypes.bfloat16
EXP = mybir.ActivationFunctionType.Exp


# ---------------------------------------------------------------- device IR
def _build_nc():
    nc = bacc.Bacc("TRN2", target_bir_lowering=False, debug=False)
    xr = nc.declare_dram_parameter("x", [BPC, S, F], BF16, isOutput=False)
    # weights [co, 128(ci-part), ci, 128(out)] so each per-co DMA slice is
    # fully contiguous on both sides (the column-sliced layout fragmented
    # into 256B packets and starved the queues)
    ws = [nc.declare_dram_parameter(f"w{m}", [FC, 128, FC, 128], BF16, isOutput=False)
          for m in range(4)]
    ball = nc.declare_dram_parameter("ball", [128, 3, FC], F32, isOutput=False)
    bor = nc.declare_dram_parameter("bor", [F], BF16, isOutput=False)
    bvr = nc.declare_dram_parameter("bvr", [F], BF16, isOutput=False)
    outr = nc.declare_dram_parameter("out", [BPC, S, F], BF16, isOutput=True)

    with tile.TileContext(nc) as tc:
        from contextlib import ExitStack
        with ExitStack() as ctx:
            xpool = ctx.enter_context(tc.tile_pool(name="xpool", bufs=2))
            # x DMAs first: the scalar queue starts streaming batch 0's
            # tokens while the constants/weights below are still being set up
            xns = []
            xq = {}
            for b in range(BPC):
                xn = xpool.tile([128, NS, F], BF16, tag="xn")
                for st in range(NS):
                    if b == 0 and st == 1:
                        # st1 rides the gpsimd queue (tiny bias DMAs ahead of
                        # it) so the first two transposes aren't serialized
                        # behind one queue at startup
                        xq[(b, st)] = (xn, st)
                        continue
                    nc.scalar.dma_start(out=xn[:ST, st, :],
                                        in_=xr[b, st * ST:(st + 1) * ST, :])
                xns.append(xn)
            const = ctx.enter_context(tc.tile_pool(name="const", bufs=1))
            big = ctx.enter_context(tc.tile_pool(name="big", bufs=2))
            qkp = ctx.enter_context(tc.tile_pool(name="qkp", bufs=2))
            stage = ctx.enter_context(tc.tile_pool(name="stage", bufs=2))
            vnp = ctx.enter_context(tc.tile_pool(name="vnp", bufs=2))
            ppool = ctx.enter_context(tc.tile_pool(name="ppool", bufs=2))
            recp = ctx.enter_context(tc.tile_pool(name="recp", bufs=2))
            # PSUM -- bank budget 8: pb 2x1 + ps 2x2 + py 2x1
            pb = ctx.enter_context(tc.tile_pool(name="pb", bufs=2, space="PSUM"))
            ps = ctx.enter_context(tc.tile_pool(name="ps", bufs=2, space="PSUM"))
            py = ctx.enter_context(tc.tile_pool(name="py", bufs=2, space="PSUM"))

            ident_f = const.tile([128, 128], F32, tag="identf")
            make_identity(nc, ident_f[:])
            ident = const.tile([128, 128], BF16, tag="ident")
            nc.vector.tensor_copy(ident[:], ident_f[:])
            ones_f = const.tile([128, 1], F32, tag="ones_f")
            nc.gpsimd.memset(ones_f[:], 1.0)
            ones_b = const.tile([128, 1], BF16, tag="ones_b")
            nc.vector.tensor_copy(ones_b[:], ones_f[:])

            bsb = const.tile([128, 3, FC], F32, tag="bsb")
            nc.gpsimd.dma_start(out=bsb[:], in_=ball[:])
            # bv/bo broadcast rows via K=1 PE matmuls (ones x bias) -- a
            # gpsimd partition_broadcast would push ~800KB through the DMA
            # queues right when the weight streams need them.
            ones_row = const.tile([1, 128], BF16, tag="ones_row")
            nc.vector.tensor_copy(ones_row[0:1, :],
                                  ones_b[0:1, 0:1].to_broadcast((1, 128)))
            bo_one = const.tile([1, F], BF16, tag="bo_one")
            nc.gpsimd.dma_start(out=bo_one[:], in_=bor.rearrange("(o f) -> o f", o=1))
            bv_one = const.tile([1, F], BF16, tag="bv_one")
            nc.gpsimd.dma_start(out=bv_one[:], in_=bvr.rearrange("(o f) -> o f", o=1))
            for (b_, st_), (xn_, _) in xq.items():
                nc.gpsimd.dma_start(out=xn_[:ST, st_, :],
                                    in_=xr[b_, st_ * ST:(st_ + 1) * ST, :])
            bo_row = const.tile([128, F], BF16, tag="bo_row")
            bv_row = const.tile([128, F], BF16, tag="bv_row")
            for row, one in ((bv_row, bv_one), (bo_row, bo_one)):
                for hf in range(2):
                    pbr = pb.tile([128, 512], F32, tag="pb",
                                  name=f"pbr{hf}")
                    nc.tensor.matmul(
                        pbr[:, :FH], ones_row[0:1, :],
                        one[0:1, hf * FH:(hf + 1) * FH], start=True, stop=True)
                    nc.vector.tensor_copy(
                        row[:, hf * FH:(hf + 1) * FH], pbr[:, :FH])

            # weight tiles [128(ci-part), co, ci, 128(out)]; w2 (v) loads
            # first -- the token-major v projection streams all of Wv.
            wsb = []
            for m in range(4):
                w = const.tile([128, FC, FC, 128], BF16, tag=f"w{m}")
                wsb.append(w)
            dma_engs = [nc.sync, nc.gpsimd]
            qcount = 0

            def loadw(m, co):
                nonlocal qcount
                eng = dma_engs[qcount % 2]
                qcount += 1
                eng.dma_start(out=wsb[m][:, co], in_=ws[m][co])

            for co in range(FC):
                loadw(2, co)
            for co in range(FC):
                loadw(0, co)
                loadw(1, co)
            for co in range(FC):
                loadw(3, co)

            # ---------------- software-pipelined emission ------------------
            # The PE executes its queue in order, so emission order IS the
            # schedule: interleave independent projection work into the
            # exp-latency windows of each attention block.
            T = [{} for _ in range(BPC)]

            def emit_tr(b, st):
                # transpose one token tile of x to feature-major
                if "xT" not in T[b]:
                    T[b]["xT"] = big.tile([128, FC, S], BF16, tag="xT",
                                          name=f"xT{b}")
                xT = T[b]["xT"]
                pt = pb.tile([128, 1024], BF16, tag="pb", name=f"pt{b}_{st}")
                for c in range(FC):
                    nc.tensor.transpose(
                        pt[:, c * 128:c * 128 + ST],
                        xns[b][:ST, st, c * 128:(c + 1) * 128],
                        ident[:ST, :ST])
                nc.vector.tensor_copy(
                    xT[:, :, st * ST:(st + 1) * ST],
                    pt[:, :FC * 128].rearrange("p (t s) -> p t s", t=FC)[:, :, :ST])

            def emit_v(b, tc):
                # token-major V projection for one token chunk (+bias +ones)
                if "vn" not in T[b]:
                    vn = vnp.tile([128, NS, FC, 2, HD + 1], BF16, tag="vn",
                                  name=f"vn{b}")
                    nc.vector.tensor_copy(
                        vn[:ST].rearrange("p t c g d -> p (t c g) d")[:, :, HD:HD + 1],
                        ones_b[:ST, 0:1].to_broadcast((ST, NS * FC * 2, 1)))
                    T[b]["vn"] = vn
                vn = T[b]["vn"]
                xT = T[b]["xT"]
                for hf in range(2):
                    pv = py.tile([128, 512], F32, tag="py", name=f"pv{b}_{tc}_{hf}")
                    for ci in range(FC):
                        nc.tensor.matmul(
                            pv[:ST, :FH],
                            xT[:, ci, tc * ST:(tc + 1) * ST],
                            wsb[2][:, hf * 3:(hf + 1) * 3, ci, :],
                            start=(ci == 0), stop=(ci == FC - 1))
                    nc.vector.tensor_add(
                        vn[:ST, tc, hf * 3:(hf + 1) * 3, :, 0:HD],
                        pv[:ST, :FH].rearrange("p (c g d) -> p c g d", c=3, g=2),
                        bv_row[:ST, hf * FH:(hf + 1) * FH].rearrange(
                            "p (c g d) -> p c g d", c=3, g=2))

            def emit_proj(b, hp, m):
                # one feature-major q/k projection chunk (12 MMs + bias)
                if "qT" not in T[b]:
                    T[b]["qT"] = qkp.tile([128, FC, S], BF16, tag="qkT",
                                          name=f"qT{b}")
                    T[b]["kT"] = qkp.tile([128, FC, S], BF16, tag="qkT",
                                          name=f"kT{b}")
                dst = T[b]["qT"] if m == 0 else T[b]["kT"]
                xT = T[b]["xT"]
                acc0 = pb.tile([128, 512], F32, tag="pb", name=f"ac0_{b}_{hp}_{m}")
                acc1 = pb.tile([128, 512], F32, tag="pb", name=f"ac1_{b}_{hp}_{m}")
                accs = (acc0, acc1)
                for ci in range(FC):
                    for h in range(2):
                        nc.tensor.matmul(
                            accs[h][:, :NH],
                            wsb[m][:, hp, ci, :],
                            xT[:, ci, h * NH:(h + 1) * NH],
                            start=(ci == 0), stop=(ci == FC - 1))
                for h in range(2):
                    nc.vector.tensor_scalar_add(
                        dst[:, hp, h * NH:(h + 1) * NH],
                        in0=accs[h][:, :NH], scalar1=bsb[:, m, hp:hp + 1])

            def emit_qk(b, hp, sh):
                # scores for both heads of the pair into the two PE
                # row-groups; one merged exp per PAIR of key tiles (4 psum
                # banks per ACTIVATE amortizes the ScalarE access overhead)
                pp = ppool.tile([128, 2, NS, NH], BF16, tag="pp",
                                name=f"pp{b}_{hp}_{sh}")
                T[b]["pp"] = pp
                qT, kT = T[b]["qT"], T[b]["kT"]
                for t5 in range(NS):
                    sc = ps.tile([128, 2, 512], F32, tag="ps",
                                 name=f"sc{b}_{hp}_{sh}_{t5}")
                    for g in range(2):
                        r0 = g * HD
                        nc.tensor.matmul(
                            sc[:ST, g, :NH],
                            kT[r0:r0 + HD, hp, t5 * ST:(t5 + 1) * ST],
                            qT[r0:r0 + HD, hp, sh * NH:(sh + 1) * NH],
                            start=True, stop=True)
                    nc.scalar.activation(
                        pp[:ST, :, t5, :], sc[:ST, :, :NH], func=EXP)

            def emit_pv(b, hp, sh):
                if "yT" not in T[b]:
                    T[b]["yT"] = big.tile([128, FC, S], BF16, tag="yT",
                                          name=f"yT{b}")
                yT = T[b]["yT"]
                vn, pp = T[b]["vn"], T[b]["pp"]
                paccs = []
                for g in range(2):
                    pacc = py.tile([128, 512], F32, tag="py",
                                   name=f"pacc{b}_{hp}_{sh}_{g}")
                    paccs.append(pacc)
                    for t5 in range(NS):
                        nc.tensor.matmul(
                            pacc[:HD + 1, :NH],
                            vn[:ST, t5, hp, g, :],
                            pp[:ST, g, t5, :],
                            start=(t5 == 0), stop=(t5 == NS - 1))
                for g in range(2):
                    r0 = g * HD
                    srow = recp.tile([1, NH], F32, tag="srow")
                    nc.vector.tensor_copy(srow[:, :], paccs[g][HD:HD + 1, :NH])
                    rec = recp.tile([1, NH], F32, tag="rec")
                    nc.vector.reciprocal_approx_fast(rec[:, :], srow[:, :])
                    rb = recp.tile([HD, NH], F32, tag="rb")
                    nc.gpsimd.partition_broadcast(rb[:, :], rec[0:1, :])
                    nc.vector.tensor_mul(
                        yT[r0:r0 + HD, hp, sh * NH:(sh + 1) * NH],
                        paccs[g][:HD, :NH], rb[:, :])

            def emit_o(b, tc):
                # token-major output projection chunk + bias row-add + store;
                # each feature half DMAs out on its own queue as soon as its
                # bias add lands
                yT = T[b]["yT"]
                on = stage.tile([128, F], BF16, tag="on", name=f"on{b}_{tc}")
                for hf in range(2):
                    po = py.tile([128, 512], F32, tag="py", name=f"po{b}_{tc}_{hf}")
                    for ci in range(FC):
                        nc.tensor.matmul(
                            po[:ST, :FH],
                            yT[:, ci, tc * ST:(tc + 1) * ST],
                            wsb[3][:, hf * 3:(hf + 1) * 3, ci, :],
                            start=(ci == 0), stop=(ci == FC - 1))
                    nc.vector.tensor_add(
                        on[:ST, hf * FH:(hf + 1) * FH],
                        po[:ST, :FH],
                        bo_row[:ST, hf * FH:(hf + 1) * FH])
                    nc.sync.dma_start(
                        out=outr[b, tc * ST:(tc + 1) * ST,
                                 hf * FH:(hf + 1) * FH],
                        in_=on[:ST, hf * FH:(hf + 1) * FH])

            def emit_pre(b):
                # x transposes interleaved with token-major V projection
                for st in range(NS):
                    emit_tr(b, st)
                    emit_v(b, st)
                emit_proj(b, 0, 0)
                emit_proj(b, 0, 1)

            def emit_body(b, post_prev, pre_next=None):
                # per head pair: attention with next pair's projections (or
                # the next batch's prologue / own output chunks) filling the
                # ScalarE exp latency windows
                for hp in range(FC):
                    emit_qk(b, hp, 0)
                    if hp < FC - 1:
                        emit_proj(b, hp + 1, 0)
                    elif pre_next:
                        pre_next(0)      # b1 transposes: pb slots are free
                    emit_pv(b, hp, 0)
                    emit_qk(b, hp, 1)
                    if hp < FC - 1:
                        emit_proj(b, hp + 1, 1)
                    else:
                        emit_o(b, 0)
                        if pre_next:
                            pre_next(1)
                    emit_pv(b, hp, 1)
                emit_o(b, 1)
                for tc in range(2, NS):
                    emit_o(b, tc)
                    if post_prev:
                        post_prev(tc - 2)

            emit_pre(0)

            def tr1(i):
                # batch 1 transposes only -- they need just a pb psum slot,
                # which batch 0's final attention no longer uses
                emit_tr(1, i)

            def post1(i):
                # v projection (py-hungry) only after batch 0's PV is done
                emit_v(1, i)
                if i + 2 < NS:
                    emit_tr(1, i + 2)

            emit_body(0, post1, tr1)
            for st in range(3, NS):
                emit_v(1, st)
            emit_proj(1, 0, 0)
            emit_proj(1, 0, 1)
            emit_body(1, None)

    nc.finalize()
    return nc


_NC_CACHE = {}


def _get_nc():
    if "nc" not in _NC_CACHE:
        _NC_CACHE["nc"] = _build_nc()
    return _NC_CACHE["nc"]


# ------------------------------------------------------------- host wrapper
def _head_major_perm():
    perm = np.empty(F, dtype=np.int64)
    i = 0
    for h1 in range(H1):
        for h2 in range(H2):
            for h3 in range(H3):
                for x in range(XD):
                    for y in range(YD):
                        for z in range(ZD):
                            a = x * H1 + h1
                            bb = y * H2 + h2
                            cc = z * H3 + h3
                            perm[i] = a * 96 + bb * 12 + cc
                            i += 1
    return perm


def _prep_inputs(inputs):
    perm = _head_major_perm()
    scale = float(HD) ** -0.5

    def kron3(w1, w2, w3):
        return np.kron(w1, np.kron(w2, w3)).astype(np.float32)

    mats = {}
    mats["w0"] = np.ascontiguousarray(
        (kron3(inputs["Wq1"], inputs["Wq2"], inputs["Wq3"])[perm, :] * scale).T)
    b0 = np.asarray(inputs["bq"]).reshape(F)[perm] * scale
    mats["w1"] = np.ascontiguousarray(
        kron3(inputs["Wk1"], inputs["Wk2"], inputs["Wk3"])[perm, :].T)
    b1 = np.asarray(inputs["bk"]).reshape(F)[perm]
    mats["w2"] = np.ascontiguousarray(
        kron3(inputs["Wv1"], inputs["Wv2"], inputs["Wv3"])[perm, :].T)
    b2 = np.asarray(inputs["bv"]).reshape(F)[perm]
    mats["w3"] = np.ascontiguousarray(
        kron3(inputs["Wo1"], inputs["Wo2"], inputs["Wo3"])[:, perm].T)
    # q/k/v biases packed [128(p), 3(m), 6(c)]: ball[p,m,c] = b_m[c*128+p]
    mats["ball"] = np.ascontiguousarray(
        np.stack([b0, b1, b2]).reshape(3, FC, 128).transpose(2, 0, 1)
    ).astype(np.float32)
    mats["bor"] = np.ascontiguousarray(
        np.asarray(inputs["bo"]).reshape(F)).astype(NPBF16)
    mats["bvr"] = np.ascontiguousarray(b2).astype(NPBF16)
    return mats


def _make_in_maps(inputs):
    mats = _prep_inputs(inputs)
    for k in ("w0", "w1", "w2", "w3"):
        mats[k] = np.ascontiguousarray(
            mats[k].reshape(FC, 128, FC, 128).transpose(2, 1, 0, 3)).astype(NPBF16)
    x = np.ascontiguousarray(
        np.asarray(inputs["x"], dtype=np.float32).reshape(B, S, F)).astype(NPBF16)
    in_maps = []
    for c in range(N_CORES):
        m = {"x": np.ascontiguousarray(x[c * BPC:(c + 1) * BPC])}
        m.update(mats)
        in_maps.append(m)
    return in_maps


def kernel(**inputs) -> np.ndarray:
    nc = _get_nc()
    in_maps = _make_in_maps(inputs)
    res = run_bass_kernel_spmd(nc, in_maps, core_ids=list(range(N_CORES)))
    out = np.concatenate(
        [np.asarray(res.results[c]["out"]).astype(np.float32)
         for c in range(N_CORES)], axis=0)
    return out.reshape(B, P1, P2, 8, 8, 12)


def run_traced(inputs, **kw):
    """test.py helper: returns (output, BassKernelResults) with trace."""
    nc = _get_nc()
    in_maps = _make_in_maps(inputs)
    res = run_bass_kernel_spmd(nc, in_maps, core_ids=list(range(N_CORES)), **kw)
    out = np.concatenate(
        [np.asarray(res.results[c]["out"]).astype(np.float32)
         for c in range(N_CORES)], axis=0)
    return out.reshape(B, P1, P2, 8, 8, 12), res


# revision 52
# speedup vs baseline: 1.0047x; 1.0047x over previous
"""Trainium2 Bass kernel for the Tucker-factorized (TLE) multi-head attention.

Strategy
--------
Data-parallel over batch: 16 batches / 8 cores = 2 batches per core; every
core runs the full per-batch pipeline (no collectives needed).

Host-side prep: the three per-mode factor matrices of each projection are
folded into one dense 768x768 Kronecker matrix.  Rows (for q/k/v) are
permuted to *head-major* order (h1,h2,h3,x,y,z) so each of the 12 heads
occupies a contiguous 64-partition block -- two heads per 128-partition
chunk.  The softmax scale 1/8 is folded into the q matrix/bias.  The o
matrix gets the inverse permutation on its columns.  Weights, x, and the
output travel as bfloat16; psum accumulation stays fp32, keeping rms error
~5e-3, well inside the 2e-2 gate.  bf16 operands run the PE at 1 cycle/row
with fast weight loads.

Device pipeline, per batch:
  1. DMA x [600,768] token-major (bf16), PE-transpose to xT [768,600].
  2. V projected *token-major* directly (xT chunks stationary, Wv moving):
     no V transposes needed; ones column appended for softmax sums.
  3. Per head pair hp: q/kT = W @ xT + b (feature-major); then per query
     half: scores for both heads of the pair issue into the two PE
     row-groups (rows 0-63 / 64-127) back-to-back and one ScalarE exp
     covers both heads' psum banks; PV via lhsT=[V|1]; normalize with a
     fast DVE reciprocal + gpsimd partition-broadcast.
  4. Output projected token-major (yT chunks stationary, Wo moving): bias
     row-add on DVE, DMA out per token chunk as bf16 (host upcasts).
"""

import numpy as np
import ml_dtypes

import concourse.bass as bass
import concourse.tile as tile
from concourse import bacc, mybir
from concourse.bass_utils import run_bass_kernel_spmd
from concourse.masks import make_identity

# ---------------------------------------------------------------- constants
N_CORES = 8
B = 16
BPC = B // N_CORES          # batches per core
P1, P2 = 25, 24
S = P1 * P2                 # 600 tokens
F = 768                     # flattened feature dim
FC = 6                      # feature chunks of 128
ST = 120                    # token tile
NS = S // ST                # 5 token tiles
NH = 300                    # half of the token axis (psum-bank sized)
FH = 384                    # half of the feature axis (psum-bank sized)
H1, H2, H3 = 2, 2, 3
XD = YD = ZD = 4
NHEADS = H1 * H2 * H3       # 12
HD = 64
F32 = mybir.dt.float32
BF16 = mybir.dt.bfloat16
NPBF16 = ml_dtypes.bfloat16
EXP = mybir.ActivationFunctionType.Exp


# ---------------------------------------------------------------- device IR
def _build_nc():
    nc = bacc.Bacc("TRN2", target_bir_lowering=False, debug=False)
    xr = nc.declare_dram_parameter("x", [BPC, S, F], BF16, isOutput=False)
    # weights [co, 128(ci-part), ci, 128(out)] so each per-co DMA slice is
    # fully contiguous on both sides (the column-sliced layout fragmented
    # into 256B packets and starved the queues)
    ws = [nc.declare_dram_parameter(f"w{m}", [FC, 128, FC, 128], BF16, isOutput=False)
          for m in range(4)]
    ball = nc.declare_dram_parameter("ball", [128, 3, FC], F32, isOutput=False)
    bor = nc.declare_dram_parameter("bor", [F], BF16, isOutput=False)
    bvr = nc.declare_dram_parameter("bvr", [F], BF16, isOutput=False)
    outr = nc.declare_dram_parameter("out", [BPC, S, F], BF16, isOutput=True)

    with tile.TileContext(nc) as tc:
        from contextlib import ExitStack
        with ExitStack() as ctx:
            xpool = ctx.enter_context(tc.tile_pool(name="xpool", bufs=2))
            # x DMAs first: the scalar queue starts streaming batch 0's
            # tokens while the constants/weights below are still being set up
            xns = []
            xq = []
            for b in range(BPC):
                xn = xpool.tile([128, NS, F], BF16, tag="xn")
                for st in range(NS):
                    if b == 0 and st == 0:
                        # first tile split across two queues: its transposes
                        # gate everything, and a cold queue moves ~41 B/ns
                        nc.scalar.dma_start(out=xn[:60, st, :],
                                            in_=xr[b, 0:60, :])
                        xq.append((xn, st))
                        continue
                    nc.scalar.dma_start(out=xn[:ST, st, :],
                                        in_=xr[b, st * ST:(st + 1) * ST, :])
                xns.append(xn)
            const = ctx.enter_context(tc.tile_pool(name="const", bufs=1))
            big = ctx.enter_context(tc.tile_pool(name="big", bufs=2))
            qkp = ctx.enter_context(tc.tile_pool(name="qkp", bufs=2))
            stage = ctx.enter_context(tc.tile_pool(name="stage", bufs=2))
            vnp = ctx.enter_context(tc.tile_pool(name="vnp", bufs=2))
            ppool = ctx.enter_context(tc.tile_pool(name="ppool", bufs=2))
            recp = ctx.enter_context(tc.tile_pool(name="recp", bufs=2))
            # PSUM -- bank budget 8: pb 2x1 + ps 2x2 + py 2x1
            pb = ctx.enter_context(tc.tile_pool(name="pb", bufs=2, space="PSUM"))
            ps = ctx.enter_context(tc.tile_pool(name="ps", bufs=2, space="PSUM"))
            py = ctx.enter_context(tc.tile_pool(name="py", bufs=2, space="PSUM"))

            ident_f = const.tile([128, 128], F32, tag="identf")
            make_identity(nc, ident_f[:])
            ident = const.tile([128, 128], BF16, tag="ident")
            nc.vector.tensor_copy(ident[:], ident_f[:])
            ones_f = const.tile([128, 1], F32, tag="ones_f")
            nc.gpsimd.memset(ones_f[:], 1.0)
            ones_b = const.tile([128, 1], BF16, tag="ones_b")
            nc.vector.tensor_copy(ones_b[:], ones_f[:])

            bsb = const.tile([128, 3, FC], F32, tag="bsb")
            nc.gpsimd.dma_start(out=bsb[:], in_=ball[:])
            # bv/bo broadcast rows via K=1 PE matmuls (ones x bias) -- a
            # gpsimd partition_broadcast would push ~800KB through the DMA
            # queues right when the weight streams need them.
            ones_row = const.tile([1, 128], BF16, tag="ones_row")
            nc.vector.tensor_copy(ones_row[0:1, :],
                                  ones_b[0:1, 0:1].to_broadcast((1, 128)))
            bo_one = const.tile([1, F], BF16, tag="bo_one")
            nc.gpsimd.dma_start(out=bo_one[:], in_=bor.rearrange("(o f) -> o f", o=1))
            bv_one = const.tile([1, F], BF16, tag="bv_one")
            nc.gpsimd.dma_start(out=bv_one[:], in_=bvr.rearrange("(o f) -> o f", o=1))
            for xn_, st_ in xq:
                nc.gpsimd.dma_start(out=xn_[60:ST, st_, :],
                                    in_=xr[0, 60:ST, :])
            bo_row = const.tile([128, F], BF16, tag="bo_row")
            bv_row = const.tile([128, F], BF16, tag="bv_row")
            for row, one in ((bv_row, bv_one), (bo_row, bo_one)):
                for hf in range(2):
                    pbr = pb.tile([128, 512], F32, tag="pb",
                                  name=f"pbr{hf}")
                    nc.tensor.matmul(
                        pbr[:, :FH], ones_row[0:1, :],
                        one[0:1, hf * FH:(hf + 1) * FH], start=True, stop=True)
                    nc.vector.tensor_copy(
                        row[:, hf * FH:(hf + 1) * FH], pbr[:, :FH])

            # weight tiles [128(ci-part), co, ci, 128(out)]; w2 (v) loads
            # first -- the token-major v projection streams all of Wv.
            wsb = []
            for m in range(4):
                w = const.tile([128, FC, FC, 128], BF16, tag=f"w{m}")
                wsb.append(w)
            dma_engs = [nc.sync, nc.gpsimd]
            qcount = 0

            def loadw(m, co):
                nonlocal qcount
                eng = dma_engs[qcount % 2]
                qcount += 1
                eng.dma_start(out=wsb[m][:, co], in_=ws[m][co])

            for co in range(FC):
                loadw(2, co)
            for co in range(FC):
                loadw(0, co)
                loadw(1, co)
            for co in range(FC):
                loadw(3, co)

            # ---------------- software-pipelined emission ------------------
            # The PE executes its queue in order, so emission order IS the
            # schedule: interleave independent projection work into the
            # exp-latency windows of each attention block.
            T = [{} for _ in range(BPC)]

            def emit_tr(b, st):
                # transpose one token tile of x to feature-major
                if "xT" not in T[b]:
                    T[b]["xT"] = big.tile([128, FC, S], BF16, tag="xT",
                                          name=f"xT{b}")
                xT = T[b]["xT"]
                pt = pb.tile([128, 1024], BF16, tag="pb", name=f"pt{b}_{st}")
                for c in range(FC):
                    nc.tensor.transpose(
                        pt[:, c * 128:c * 128 + ST],
                        xns[b][:ST, st, c * 128:(c + 1) * 128],
                        ident[:ST, :ST])
                nc.vector.tensor_copy(
                    xT[:, :, st * ST:(st + 1) * ST],
                    pt[:, :FC * 128].rearrange("p (t s) -> p t s", t=FC)[:, :, :ST])

            def emit_v(b, tc):
                # token-major V projection for one token chunk (+bias +ones)
                if "vn" not in T[b]:
                    vn = vnp.tile([128, NS, FC, 2, HD + 1], BF16, tag="vn",
                                  name=f"vn{b}")
                    nc.vector.tensor_copy(
                        vn[:ST].rearrange("p t c g d -> p (t c g) d")[:, :, HD:HD + 1],
                        ones_b[:ST, 0:1].to_broadcast((ST, NS * FC * 2, 1)))
                    T[b]["vn"] = vn
                vn = T[b]["vn"]
                xT = T[b]["xT"]
                for hf in range(2):
                    pv = py.tile([128, 512], F32, tag="py", name=f"pv{b}_{tc}_{hf}")
                    for ci in range(FC):
                        nc.tensor.matmul(
                            pv[:ST, :FH],
                            xT[:, ci, tc * ST:(tc + 1) * ST],
                            wsb[2][:, hf * 3:(hf + 1) * 3, ci, :],
                            start=(ci == 0), stop=(ci == FC - 1))
                    nc.vector.tensor_add(
                        vn[:ST, tc, hf * 3:(hf + 1) * 3, :, 0:HD],
                        pv[:ST, :FH].rearrange("p (c g d) -> p c g d", c=3, g=2),
                        bv_row[:ST, hf * FH:(hf + 1) * FH].rearrange(
                            "p (c g d) -> p c g d", c=3, g=2))

            def emit_proj(b, hp, m):
                # one feature-major q/k projection chunk (12 MMs + bias)
                if "qT" not in T[b]:
                    T[b]["qT"] = qkp.tile([128, FC, S], BF16, tag="qkT",
                                          name=f"qT{b}")
                    T[b]["kT"] = qkp.tile([128, FC, S], BF16, tag="qkT",
                                          name=f"kT{b}")
                dst = T[b]["qT"] if m == 0 else T[b]["kT"]
                xT = T[b]["xT"]
                acc0 = pb.tile([128, 512], F32, tag="pb", name=f"ac0_{b}_{hp}_{m}")
                acc1 = pb.tile([128, 512], F32, tag="pb", name=f"ac1_{b}_{hp}_{m}")
                accs = (acc0, acc1)
                for ci in range(FC):
                    for h in range(2):
                        nc.tensor.matmul(
                            accs[h][:, :NH],
                            wsb[m][:, hp, ci, :],
                            xT[:, ci, h * NH:(h + 1) * NH],
                            start=(ci == 0), stop=(ci == FC - 1))
                for h in range(2):
                    nc.vector.tensor_scalar_add(
                        dst[:, hp, h * NH:(h + 1) * NH],
                        in0=accs[h][:, :NH], scalar1=bsb[:, m, hp:hp + 1])

            def emit_qk(b, hp, sh):
                # scores for both heads of the pair into the two PE
                # row-groups; one merged exp per PAIR of key tiles (4 psum
                # banks per ACTIVATE amortizes the ScalarE access overhead)
                pp = ppool.tile([128, 2, NS, NH], BF16, tag="pp",
                                name=f"pp{b}_{hp}_{sh}")
                T[b]["pp"] = pp
                qT, kT = T[b]["qT"], T[b]["kT"]
                for t5 in range(NS):
                    sc = ps.tile([128, 2, 512], F32, tag="ps",
                                 name=f"sc{b}_{hp}_{sh}_{t5}")
                    for g in range(2):
                        r0 = g * HD
                        nc.tensor.matmul(
                            sc[:ST, g, :NH],
                            kT[r0:r0 + HD, hp, t5 * ST:(t5 + 1) * ST],
                            qT[r0:r0 + HD, hp, sh * NH:(sh + 1) * NH],
                            start=True, stop=True)
                    nc.scalar.activation(
                        pp[:ST, :, t5, :], sc[:ST, :, :NH], func=EXP)

            def emit_pv(b, hp, sh):
                if "yT" not in T[b]:
                    T[b]["yT"] = big.tile([128, FC, S], BF16, tag="yT",
                                          name=f"yT{b}")
                yT = T[b]["yT"]
                vn, pp = T[b]["vn"], T[b]["pp"]
                paccs = []
                for g in range(2):
                    pacc = py.tile([128, 512], F32, tag="py",
                                   name=f"pacc{b}_{hp}_{sh}_{g}")
                    paccs.append(pacc)
                    for t5 in range(NS):
                        nc.tensor.matmul(
                            pacc[:HD + 1, :NH],
                            vn[:ST, t5, hp, g, :],
                            pp[:ST, g, t5, :],
                            start=(t5 == 0), stop=(t5 == NS - 1))
                for g in range(2):
                    r0 = g * HD
                    srow = recp.tile([1, NH], F32, tag="srow")
                    nc.vector.tensor_copy(srow[:, :], paccs[g][HD:HD + 1, :NH])
                    rec = recp.tile([1, NH], F32, tag="rec")
                    nc.vector.reciprocal_approx_fast(rec[:, :], srow[:, :])
                    rb = recp.tile([HD, NH], F32, tag="rb")
                    nc.gpsimd.partition_broadcast(rb[:, :], rec[0:1, :])
                    nc.vector.tensor_mul(
                        yT[r0:r0 + HD, hp, sh * NH:(sh + 1) * NH],
                        paccs[g][:HD, :NH], rb[:, :])

            def emit_o(b, tc):
                # token-major output projection chunk + bias row-add + store;
                # each feature half DMAs out on its own queue as soon as its
                # bias add lands
                yT = T[b]["yT"]
                on = stage.tile([128, F], BF16, tag="on", name=f"on{b}_{tc}")
                for hf in range(2):
                    po = py.tile([128, 512], F32, tag="py", name=f"po{b}_{tc}_{hf}")
                    for ci in range(FC):
                        nc.tensor.matmul(
                            po[:ST, :FH],
                            yT[:, ci, tc * ST:(tc + 1) * ST],
                            wsb[3][:, hf * 3:(hf + 1) * 3, ci, :],
                            start=(ci == 0), stop=(ci == FC - 1))
                    nc.vector.tensor_add(
                        on[:ST, hf * FH:(hf + 1) * FH],
                        po[:ST, :FH],
                        bo_row[:ST, hf * FH:(hf + 1) * FH])
                    nc.sync.dma_start(
                        out=outr[b, tc * ST:(tc + 1) * ST,
                                 hf * FH:(hf + 1) * FH],
                        in_=on[:ST, hf * FH:(hf + 1) * FH])

            def emit_pre(b):
                # x transposes interleaved with token-major V projection
                for st in range(NS):
                    emit_tr(b, st)
                    emit_v(b, st)
                emit_proj(b, 0, 0)
                emit_proj(b, 0, 1)

            def emit_body(b, post_prev, pre_next=None):
                # per head pair: attention with next pair's projections (or
                # the next batch's prologue / own output chunks) filling the
                # ScalarE exp latency windows
                for hp in range(FC):
                    emit_qk(b, hp, 0)
                    if hp < FC - 1:
                        emit_proj(b, hp + 1, 0)
                    elif pre_next:
                        pre_next(0)      # b1 transposes: pb slots are free
                    emit_pv(b, hp, 0)
                    emit_qk(b, hp, 1)
                    if hp < FC - 1:
                        emit_proj(b, hp + 1, 1)
                    else:
                        emit_o(b, 0)
                        if pre_next:
                            pre_next(1)
                    emit_pv(b, hp, 1)
                emit_o(b, 1)
                for tc in range(2, NS):
                    emit_o(b, tc)
                    if post_prev:
                        post_prev(tc - 2)

            emit_pre(0)

            def tr1(i):
                # batch 1 transposes only -- they need just a pb psum slot,
                # which batch 0's final attention no longer uses
                emit_tr(1, i)

            def post1(i):
                # v projection (py-hungry) only after batch 0's PV is done
                emit_v(1, i)
                if i + 2 < NS:
                    emit_tr(1, i + 2)

            emit_body(0, post1, tr1)
            for st in range(3, NS):
                emit_v(1, st)
            emit_proj(1, 0, 0)
            emit_proj(1, 0, 1)
            emit_body(1, None)

    nc.finalize()
    return nc


_NC_CACHE = {}


def _get_nc():
    if "nc" not in _NC_CACHE:
        _NC_CACHE["nc"] = _build_nc()
    return _NC_CACHE["nc"]


# ------------------------------------------------------------- host wrapper
def _head_major_perm():
    perm = np.empty(F, dtype=np.int64)
    i = 0
    for h1 in range(H1):
        for h2 in range(H2):
            for h3 in range(H3):
                for x in range(XD):
                    for y in range(YD):
                        for z in range(ZD):
                            a = x * H1 + h1
                            bb = y * H2 + h2
                            cc = z * H3 + h3
                            perm[i] = a * 96 + bb * 12 + cc
                            i += 1
    return perm


def _prep_inputs(inputs):
    perm = _head_major_perm()
    scale = float(HD) ** -0.5

    def kron3(w1, w2, w3):
        return np.kron(w1, np.kron(w2, w3)).astype(np.float32)

    mats = {}
    mats["w0"] = np.ascontiguousarray(
        (kron3(inputs["Wq1"], inputs["Wq2"], inputs["Wq3"])[perm, :] * scale).T)
    b0 = np.asarray(inputs["bq"]).reshape(F)[perm] * scale
    mats["w1"] = np.ascontiguousarray(
        kron3(inputs["Wk1"], inputs["Wk2"], inputs["Wk3"])[perm, :].T)
    b1 = np.asarray(inputs["bk"]).reshape(F)[perm]
    mats["w2"] = np.ascontiguousarray(
        kron3(inputs["Wv1"], inputs["Wv2"], inputs["Wv3"])[perm, :].T)
    b2 = np.asarray(inputs["bv"]).reshape(F)[perm]
    mats["w3"] = np.ascontiguousarray(
        kron3(inputs["Wo1"], inputs["Wo2"], inputs["Wo3"])[:, perm].T)
    # q/k/v biases packed [128(p), 3(m), 6(c)]: ball[p,m,c] = b_m[c*128+p]
    mats["ball"] = np.ascontiguousarray(
        np.stack([b0, b1, b2]).reshape(3, FC, 128).transpose(2, 0, 1)
    ).astype(np.float32)
    mats["bor"] = np.ascontiguousarray(
        np.asarray(inputs["bo"]).reshape(F)).astype(NPBF16)
    mats["bvr"] = np.ascontiguousarray(b2).astype(NPBF16)
    return mats


def _make_in_maps(inputs):
    mats = _prep_inputs(inputs)
    for k in ("w0", "w1", "w2", "w3"):
        mats[k] = np.ascontiguousarray(
            mats[k].reshape(FC, 128, FC, 128).transpose(2, 1, 0, 3)).astype(NPBF16)
    x = np.ascontiguousarray(
        np.asarray(inputs["x"], dtype=np.float32).reshape(B, S, F)).astype(NPBF16)
    in_maps = []
    for c in range(N_CORES):
        m = {"x": np.ascontiguousarray(x[c * BPC:(c + 1) * BPC])}
        m.update(mats)
        in_maps.append(m)
    return in_maps


def kernel(**inputs) -> np.ndarray:
    nc = _get_nc()
    in_maps = _make_in_maps(inputs)
    res = run_bass_kernel_spmd(nc, in_maps, core_ids=list(range(N_CORES)))
    out = np.concatenate(
        [np.asarray(res.results[c]["out"]).astype(np.float32)
         for c in range(N_CORES)], axis=0)
    return out.reshape(B, P1, P2, 8, 8, 12)


def run_traced(inputs, **kw):
    """test.py helper: returns (output, BassKernelResults) with trace."""
    nc = _get_nc()
    in_maps = _make_in_maps(inputs)
    res = run_bass_kernel_spmd(nc, in_maps, core_ids=list(range(N_CORES)), **kw)
    out = np.concatenate(
        [np.asarray(res.results[c]["out"]).astype(np.float32)
         for c in range(N_CORES)], axis=0)
    return out.reshape(B, P1, P2, 8, 8, 12), res
